# revision 7
# baseline (speedup 1.0000x reference)
"""Trainium2 Bass kernel for the online-k-means "CentroidModule" (vq_codebook).

Problem (hardcoded shapes):
  batch [64, 1024, 256] f32, protos [512, 256] f32,
  protoSums [512, 256] f32, protoCounts [512] f32.
Returns (protos_new, batchSums, closestCounts, closest) like the reference.

Strategy (data-parallel over B*T across 8 cores, per the sharding hint):
  Each core gets 8192 tokens. Per 128-token chunk:
    1. DMA the [128, 256] chunk (natural layout, token-major).
    2. PE-transpose it (fp32, via identity) -> x^T tiles [d, t] in PSUM,
       evacuated to SBUF by the scalar engine.
    3. PE fp32 matmul: scores_psum[t, k] = x^T.T @ protosT (2 accum steps).
    4. DVE: add replicated (-0.5*||c||^2) row (PSUM->SBUF), max8, max_index
       -> per-token argmax (= argmin distance) written into a persistent
       uint32 tile, 8 cols per chunk.
    5. GPSIMD: onehot[t, k] = (score >= max) as f32 (exact: only the max
       position matches; ties have ~zero probability in f32 data).
    6. PE fp32r matmuls accumulate batchSums^T[d, k] += x_chunk^T @ onehot
       into 2 persistent PSUM banks across all 64 chunks.
  Host: gathers per-core outputs, sums batchSums over cores, bincounts
  closest, and does the final (protoSums+batchSums)/max(counts,1) division.
"""

import os
from contextlib import ExitStack

import numpy as np

import concourse.bass as bass
import concourse.mybir as mybir
import concourse.tile as tile
from concourse import bacc
from concourse.bass_utils import run_bass_kernel_spmd

N_CORES = 8
B, T, D, K = 64, 1024, 256, 512
TOK = (B * T) // N_CORES  # 8192 tokens per core
P = 128
NCH = TOK // P  # 64 chunks per core

F32 = mybir.dt.float32
F32R = mybir.dt.float32r
U32 = mybir.dt.uint32


def _build_module() -> bass.Bass:
    nc = bacc.Bacc(
        "TRN2",
        target_bir_lowering=False,
        debug=False,
        num_devices=N_CORES,
    )

    xb = nc.declare_dram_parameter("xb", [TOK, D], F32R, isOutput=False)
    # one packed constant blob -> one DMA -> one semaphore for all consts:
    # cols [0:K) = protosT[0:128], [K:2K) = protosT[128:256],
    # [2K:3K) = -0.5*||c||^2 replicated, [3K:3K+P) = identity
    cst = nc.declare_dram_parameter("cst", [P, 3 * K + P], F32, isOutput=False)
    cl8 = nc.declare_dram_parameter("cl8", [P, NCH * 8], U32, isOutput=True)
    ssum = nc.declare_dram_parameter("ssum", [2, P, K], F32, isOutput=True)

    with ExitStack() as ctx:
        tc = ctx.enter_context(tile.TileContext(nc))
        const = ctx.enter_context(tc.tile_pool(name="const", bufs=1))
        xpool = ctx.enter_context(tc.tile_pool(name="x", bufs=3))
        btpool = ctx.enter_context(tc.tile_pool(name="bt", bufs=3))
        spool = ctx.enter_context(tc.tile_pool(name="scores", bufs=3))
        ohpool = ctx.enter_context(tc.tile_pool(name="oh", bufs=3))
        mxpool = ctx.enter_context(tc.tile_pool(name="mx", bufs=4))
        pbt = ctx.enter_context(tc.tile_pool(name="pbt", bufs=2, space="PSUM"))
        pscore = ctx.enter_context(tc.tile_pool(name="pscore", bufs=3, space="PSUM"))
        pacc = ctx.enter_context(tc.tile_pool(name="pacc", bufs=1, space="PSUM"))

        # Persistent constants (single packed load)
        cst_sb = const.tile([P, 3 * K + P], F32)
        nc.sync.dma_start(cst_sb[:], cst[:])
        pT_sb = cst_sb  # [:, 0:K] and [:, K:2K]
        csq_sb = cst_sb[:, 2 * K : 3 * K]
        id_sb = cst_sb[:, 3 * K : 3 * K + P]
        cl8_sb = const.tile([P, NCH * 8], U32)

        # Persistent PSUM accumulators for batchSums^T (two d-blocks)
        acc0 = pacc.tile([P, K], F32)
        acc1 = pacc.tile([P, K], F32)


        for i in range(NCH):
            bn = xpool.tile([P, D], F32R)
            nc.sync.dma_start(bn[:], xb[i * P : (i + 1) * P, :])

            # x^T via PE transpose (exact for fp32)
            pt_ps = pbt.tile([P, D], F32)
            nc.tensor.transpose(pt_ps[:, 0:P], bn[:, 0:P].bitcast(F32), id_sb)
            nc.tensor.transpose(pt_ps[:, P:D], bn[:, P:D].bitcast(F32), id_sb)
            bT = btpool.tile([P, D], F32)
            nc.scalar.copy(bT[:], pt_ps[:])

            # scores[t, k] = sum_d x[t,d] * c[k,d]  (full fp32)
            ps = pscore.tile([P, K], F32)
            nc.tensor.matmul(ps[:], bT[:, 0:P], pT_sb[:, 0:K], start=True, stop=False)
            nc.tensor.matmul(ps[:], bT[:, P:D], pT_sb[:, K : 2 * K], start=False, stop=True)

            # + (-0.5*||c||^2), PSUM -> SBUF
            ssb = spool.tile([P, K], F32)
            nc.vector.tensor_tensor(ssb[:], ps[:], csq_sb, mybir.AluOpType.add)

            # per-token max and its index (argmin of distance)
            mx = mxpool.tile([P, 8], F32)
            nc.vector.max(mx[:], ssb[:])
            nc.vector.max_index(cl8_sb[:, i * 8 : (i + 1) * 8], mx[:], ssb[:])

            # onehot[t, k] = (score >= max), written natively as fp32r
            # (exact: values are only 0.0 / 1.0)
            oh = ohpool.tile([P, K], F32R)
            nc.gpsimd.tensor_scalar(
                oh[:], ssb[:], mx[:, 0:1], None, mybir.AluOpType.is_ge
            )

            # batchSums^T[d, k] += x_chunk[t, d]^T @ onehot[t, k]  (fp32r)
            st, sp = (i == 0), (i == NCH - 1)
            nc.tensor.matmul(acc0[:], bn[:, 0:P], oh[:], start=st, stop=sp)
            nc.tensor.matmul(acc1[:], bn[:, P:D], oh[:], start=st, stop=sp)

        # Evacuate accumulators and indices
        out0 = const.tile([P, K], F32, tag="out0")
        out1 = const.tile([P, K], F32, tag="out1")
        nc.scalar.copy(out0[:], acc0[:])
        nc.scalar.copy(out1[:], acc1[:])
        nc.sync.dma_start(ssum[0], out0[:])
        nc.sync.dma_start(ssum[1], out1[:])
        nc.sync.dma_start(cl8[:], cl8_sb[:])

    nc.compile()
    return nc


_CACHE: dict = {}


def _get_module() -> bass.Bass:
    if "nc" not in _CACHE:
        _CACHE["nc"] = _build_module()
    return _CACHE["nc"]


def _run(batch, protos, protoSums, protoCounts, trace=False):
    batch = np.ascontiguousarray(np.asarray(batch), dtype=np.float32)
    protos = np.ascontiguousarray(np.asarray(protos), dtype=np.float32)
    protoSums = np.asarray(protoSums, dtype=np.float32)
    protoCounts = np.asarray(protoCounts, dtype=np.float32)

    x = batch.reshape(B * T, D)
    shards = x.reshape(N_CORES, TOK, D)
    pT_h = protos.T  # [D, K]
    csq = np.sum(protos * protos, axis=1)  # f32, like the reference's c_sq
    cst_h = np.empty((P, 3 * K + P), dtype=np.float32)
    cst_h[:, 0:K] = pT_h[0:P, :]
    cst_h[:, K : 2 * K] = pT_h[P:D, :]
    cst_h[:, 2 * K : 3 * K] = (-0.5 * csq)[None, :]
    cst_h[:, 3 * K :] = np.eye(P, dtype=np.float32)

    in_maps = [
        {"xb": np.ascontiguousarray(shards[c]), "cst": cst_h}
        for c in range(N_CORES)
    ]

    nc = _get_module()
    res = run_bass_kernel_spmd(nc, in_maps, list(range(N_CORES)), trace=trace)

    closest = np.empty((N_CORES, TOK), dtype=np.int32)
    bsumsT = np.zeros((D, K), dtype=np.float64)
    for c in range(N_CORES):
        out = res.results[c]
        idx = out["cl8"].reshape(P, NCH, 8)[:, :, 0]  # [p, chunk]
        closest[c] = idx.T.reshape(TOK).astype(np.int64).astype(np.int32)
        bsumsT += out["ssum"].reshape(2 * P, K).astype(np.float64)

    closest_full = closest.reshape(B, T)
    counts = np.bincount(closest.reshape(-1), minlength=K).astype(np.float32)
    batchSums = bsumsT.T.astype(np.float32)  # [K, D]

    newSums = protoSums + batchSums
    newCounts = protoCounts + counts
    protos_new = newSums / np.maximum(newCounts[:, None], 1.0)
    return (protos_new, batchSums, counts, closest_full), res


def kernel(batch, protos, protoSums, protoCounts):
    out, _ = _run(
        batch,
        protos,
        protoSums,
        protoCounts,
        trace=bool(int(os.environ.get("KERNEL_TRACE", "0"))),
    )
    return out


# revision 8
# speedup vs baseline: 2.8708x; 2.8708x over previous
"""Trainium2 Bass kernel for the online-k-means "CentroidModule" (vq_codebook).

Problem (hardcoded shapes):
  batch [64, 1024, 256] f32, protos [512, 256] f32,
  protoSums [512, 256] f32, protoCounts [512] f32.
Returns (protos_new, batchSums, closestCounts, closest) like the reference.

Strategy (data-parallel over B*T across 8 cores, per the sharding hint):
  Each core gets 8192 tokens. Per 128-token chunk:
    1. DMA the [128, 256] chunk (natural layout, token-major).
    2. PE-transpose it (fp32, via identity) -> x^T tiles [d, t] in PSUM,
       evacuated to SBUF by the scalar engine.
    3. PE fp32 matmul: scores_psum[t, k] = x^T.T @ protosT (2 accum steps).
    4. DVE: add replicated (-0.5*||c||^2) row (PSUM->SBUF), max8, max_index
       -> per-token argmax (= argmin distance) written into a persistent
       uint32 tile, 8 cols per chunk.
    5. GPSIMD: onehot[t, k] = (score >= max) as f32 (exact: only the max
       position matches; ties have ~zero probability in f32 data).
    6. PE fp32r matmuls accumulate batchSums^T[d, k] += x_chunk^T @ onehot
       into 2 persistent PSUM banks across all 64 chunks.
  Host: gathers per-core outputs, sums batchSums over cores, bincounts
  closest, and does the final (protoSums+batchSums)/max(counts,1) division.
"""

import os
from contextlib import ExitStack

import numpy as np

import concourse.bass as bass
import concourse.mybir as mybir
import concourse.tile as tile
from concourse import bacc
from concourse.bass_utils import run_bass_kernel_spmd

N_CORES = 8
B, T, D, K = 64, 1024, 256, 512
TOK = (B * T) // N_CORES  # 8192 tokens per core
P = 128
NCH = TOK // P  # 64 chunks per core

F32 = mybir.dt.float32
F32R = mybir.dt.float32r
U32 = mybir.dt.uint32


def _build_module() -> bass.Bass:
    nc = bacc.Bacc(
        "TRN2",
        target_bir_lowering=False,
        debug=False,
        num_devices=N_CORES,
    )

    xb = nc.declare_dram_parameter("xb", [TOK, D], F32R, isOutput=False)
    # one packed constant blob -> one DMA -> one semaphore for all consts:
    # cols [0:K) = protosT[0:128], [K:2K) = protosT[128:256],
    # [2K:3K) = -0.5*||c||^2 replicated, [3K:3K+P) = identity
    cst = nc.declare_dram_parameter("cst", [P, 3 * K + P], F32, isOutput=False)
    cl8 = nc.declare_dram_parameter("cl8", [P, NCH * 8], U32, isOutput=True)
    ssum = nc.declare_dram_parameter("ssum", [2, P, K], F32, isOutput=True)

    with ExitStack() as ctx:
        tc = ctx.enter_context(tile.TileContext(nc))
        const = ctx.enter_context(tc.tile_pool(name="const", bufs=1))
        xpool = ctx.enter_context(tc.tile_pool(name="x", bufs=3))
        btpool = ctx.enter_context(tc.tile_pool(name="bt", bufs=3))
        spool = ctx.enter_context(tc.tile_pool(name="scores", bufs=3))
        ohpool = ctx.enter_context(tc.tile_pool(name="oh", bufs=3))
        mxpool = ctx.enter_context(tc.tile_pool(name="mx", bufs=4))
        pbt = ctx.enter_context(tc.tile_pool(name="pbt", bufs=2, space="PSUM"))
        pscore = ctx.enter_context(tc.tile_pool(name="pscore", bufs=3, space="PSUM"))
        pacc = ctx.enter_context(tc.tile_pool(name="pacc", bufs=1, space="PSUM"))

        # Persistent constants (single packed load)
        cst_sb = const.tile([P, 3 * K + P], F32)
        nc.sync.dma_start(cst_sb[:], cst[:])
        pT_sb = cst_sb  # [:, 0:K] and [:, K:2K]
        csq_sb = cst_sb[:, 2 * K : 3 * K]
        id_sb = cst_sb[:, 3 * K : 3 * K + P]
        cl8_sb = const.tile([P, NCH * 8], U32)

        # Persistent PSUM accumulators for batchSums^T (two d-blocks)
        acc0 = pacc.tile([P, K], F32)
        acc1 = pacc.tile([P, K], F32)


        for i in range(NCH):
            bn = xpool.tile([P, D], F32R)
            nc.sync.dma_start(bn[:], xb[i * P : (i + 1) * P, :])

            # x^T via PE transpose (exact for fp32)
            pt_ps = pbt.tile([P, D], F32)
            nc.tensor.transpose(pt_ps[:, 0:P], bn[:, 0:P].bitcast(F32), id_sb)
            nc.tensor.transpose(pt_ps[:, P:D], bn[:, P:D].bitcast(F32), id_sb)
            bT = btpool.tile([P, D], F32)
            nc.scalar.copy(bT[:], pt_ps[:])

            # scores[t, k] = sum_d x[t,d] * c[k,d]  (full fp32)
            ps = pscore.tile([P, K], F32)
            nc.tensor.matmul(ps[:], bT[:, 0:P], pT_sb[:, 0:K], start=True, stop=False)
            nc.tensor.matmul(ps[:], bT[:, P:D], pT_sb[:, K : 2 * K], start=False, stop=True)

            # + (-0.5*||c||^2), PSUM -> SBUF
            ssb = spool.tile([P, K], F32)
            nc.vector.tensor_tensor(ssb[:], ps[:], csq_sb, mybir.AluOpType.add)

            # per-token max and its index (argmin of distance)
            mx = mxpool.tile([P, 8], F32)
            nc.vector.max(mx[:], ssb[:])
            nc.vector.max_index(cl8_sb[:, i * 8 : (i + 1) * 8], mx[:], ssb[:])

            # onehot[t, k] = (score >= max), written natively as fp32r
            # (exact: values are only 0.0 / 1.0)
            oh = ohpool.tile([P, K], F32R)
            nc.vector.tensor_scalar(
                oh[:], ssb[:], mx[:, 0:1], None, mybir.AluOpType.is_ge
            )

            # batchSums^T[d, k] += x_chunk[t, d]^T @ onehot[t, k]  (fp32r)
            st, sp = (i == 0), (i == NCH - 1)
            nc.tensor.matmul(acc0[:], bn[:, 0:P], oh[:], start=st, stop=sp)
            nc.tensor.matmul(acc1[:], bn[:, P:D], oh[:], start=st, stop=sp)

        # Evacuate accumulators and indices
        out0 = const.tile([P, K], F32, tag="out0")
        out1 = const.tile([P, K], F32, tag="out1")
        nc.scalar.copy(out0[:], acc0[:])
        nc.scalar.copy(out1[:], acc1[:])
        nc.sync.dma_start(ssum[0], out0[:])
        nc.sync.dma_start(ssum[1], out1[:])
        nc.sync.dma_start(cl8[:], cl8_sb[:])

    nc.compile()
    return nc


_CACHE: dict = {}


def _get_module() -> bass.Bass:
    if "nc" not in _CACHE:
        _CACHE["nc"] = _build_module()
    return _CACHE["nc"]


def _run(batch, protos, protoSums, protoCounts, trace=False):
    batch = np.ascontiguousarray(np.asarray(batch), dtype=np.float32)
    protos = np.ascontiguousarray(np.asarray(protos), dtype=np.float32)
    protoSums = np.asarray(protoSums, dtype=np.float32)
    protoCounts = np.asarray(protoCounts, dtype=np.float32)

    x = batch.reshape(B * T, D)
    shards = x.reshape(N_CORES, TOK, D)
    pT_h = protos.T  # [D, K]
    csq = np.sum(protos * protos, axis=1)  # f32, like the reference's c_sq
    cst_h = np.empty((P, 3 * K + P), dtype=np.float32)
    cst_h[:, 0:K] = pT_h[0:P, :]
    cst_h[:, K : 2 * K] = pT_h[P:D, :]
    cst_h[:, 2 * K : 3 * K] = (-0.5 * csq)[None, :]
    cst_h[:, 3 * K :] = np.eye(P, dtype=np.float32)

    in_maps = [
        {"xb": np.ascontiguousarray(shards[c]), "cst": cst_h}
        for c in range(N_CORES)
    ]

    nc = _get_module()
    res = run_bass_kernel_spmd(nc, in_maps, list(range(N_CORES)), trace=trace)

    closest = np.empty((N_CORES, TOK), dtype=np.int32)
    bsumsT = np.zeros((D, K), dtype=np.float64)
    for c in range(N_CORES):
        out = res.results[c]
        idx = out["cl8"].reshape(P, NCH, 8)[:, :, 0]  # [p, chunk]
        closest[c] = idx.T.reshape(TOK).astype(np.int64).astype(np.int32)
        bsumsT += out["ssum"].reshape(2 * P, K).astype(np.float64)

    closest_full = closest.reshape(B, T)
    counts = np.bincount(closest.reshape(-1), minlength=K).astype(np.float32)
    batchSums = bsumsT.T.astype(np.float32)  # [K, D]

    newSums = protoSums + batchSums
    newCounts = protoCounts + counts
    protos_new = newSums / np.maximum(newCounts[:, None], 1.0)
    return (protos_new, batchSums, counts, closest_full), res


def kernel(batch, protos, protoSums, protoCounts):
    out, _ = _run(
        batch,
        protos,
        protoSums,
        protoCounts,
        trace=bool(int(os.environ.get("KERNEL_TRACE", "0"))),
    )
    return out
